# revision 16
# baseline (speedup 1.0000x reference)
"""Multi-head self-attention (causal) Trainium2 Bass kernel, 8-core SPMD. v2.

Sharding: 8 cores = 2 batches x 4 head-groups (3 heads each), as v1.

v2 restructure vs v1 (cost model charges matmuls by output free-dim columns):
  - AV computed in STRAIGHT form out[q,d], kt-major: after each kt's exp,
    every live (q-tile, head) chain of the chunk accumulates that kt's
    65-col contribution (64 d + ones-denominator). Halves AV's PE columns
    vs the transposed form and spreads them evenly across the exp stream.
    The 12 concurrent chains share one 2-bank PSUM tile, zeroed by a memset
    and accumulated with start=False (PSUM auto-zero is 2KB-granular, so
    independent start=True chains cannot share banks).
  - Per-q-tile output normalized via per-partition scalar recip, then
    PE-transposed (identity rhs) to [d, q] for the projection; h0/h1 share
    one [128,128] transpose.
  - softmax exp batched 3 heads per ACT instruction ([128,3,256] PSUM score
    tiles) - 1/3 the ACT access-latency overhead.
  - output partials stored bf16 (half the write DMA traffic).
  - deficit-paced PE fill queue: next-chunk QKV (K/V per-ktile late-split
    for the last chunk) and per-tile transpose+projection tails are
    deferred into ACT-paced slots so PE never starves.

PSUM budget (8 banks): scores [128,3,256]x2 = 4, AV accum = 2, pmm x2 = 2.
"""

import numpy as np
import ml_dtypes

S = 2048          # sequence length
D = 768           # model dim
HD = 64           # head dim
HPC = 3           # heads per core
NCORES = 8
P = 128           # partitions
CT = D // P       # 6 contraction tiles over model dim
KT = S // P       # 16 key tiles
QC = 512          # query chunk
NQC = S // QC     # 4 query chunks
HF = 256          # score-psum half-chunk (2 banks for [128,3,HF] f32)

_BF = ml_dtypes.bfloat16

_cache = {}


def _build_nc():
    import concourse.bass as bass
    import concourse.mybir as mybir
    import concourse.tile as tile
    from concourse import bacc
    from contextlib import ExitStack
    from collections import deque

    bf = mybir.dt.bfloat16
    f32 = mybir.dt.float32

    nc = bacc.Bacc()
    xT = nc.declare_dram_parameter("xT", [D, S], bf, isOutput=False)
    # 3 lhsT slots per c-tile: 0=[Wk0|Wk1] 1=[Wq0|Wq1] 2=[Wk2|Wq2]
    w_qk = nc.declare_dram_parameter("w_qk", [D, 3, P], bf, isOutput=False)
    w_v = nc.declare_dram_parameter("w_v", [D, HPC * HD], bf, isOutput=False)
    # col 0: [bq_h0 | bq_h1]; col 1: rows 64:128 = bq_h2
    bq = nc.declare_dram_parameter("bq", [P, 2], f32, isOutput=False)
    # rows: W_proj rows of h0, h1, h2 stacked
    w_p = nc.declare_dram_parameter("w_p", [HPC * HD, D], bf, isOutput=False)
    mask = nc.declare_dram_parameter("mask", [P, P], bf, isOutput=False)
    ident = nc.declare_dram_parameter("ident", [P, P], bf, isOutput=False)
    out_p = nc.declare_dram_parameter("out_p", [S, D], bf, isOutput=True)

    Exp = mybir.ActivationFunctionType.Exp

    import os

    with tile.TileContext(nc) as tc, ExitStack() as ctx:
        singles = ctx.enter_context(tc.tile_pool(name="singles", bufs=1))
        ps3 = ctx.enter_context(tc.tile_pool(name="ps3", bufs=2, space="PSUM"))
        pav = ctx.enter_context(tc.tile_pool(name="pav", bufs=2, space="PSUM"))
        pmm = ctx.enter_context(tc.tile_pool(name="pmm", bufs=2, space="PSUM"))
        ptp = ctx.enter_context(tc.tile_pool(
            name="ptp", bufs=int(os.environ.get("K_PTB", "18"))))
        import os as _os
        np_pool = ctx.enter_context(tc.tile_pool(
            name="npool", bufs=int(_os.environ.get("K_NPB", "3"))))
        ao_pool = ctx.enter_context(tc.tile_pool(
            name="aop", bufs=int(_os.environ.get("K_AOB", "8"))))
        at_pool = ctx.enter_context(tc.tile_pool(
            name="atp", bufs=int(_os.environ.get("K_ATB", "8"))))
        outs_pool = ctx.enter_context(tc.tile_pool(
            name="outs", bufs=int(_os.environ.get("K_OBB", "6"))))

        # ---- persistent SBUF ----
        xT_s = singles.tile([P, CT, S], bf)
        wqk_s = singles.tile([P, CT, 3, P], bf)
        wv_s = singles.tile([P, CT, HPC * HD], bf)
        bq_s = singles.tile([P, 2], f32)
        mask_s = singles.tile([P, P], bf)
        ident_s = singles.tile([P, P], bf)
        wpa_s = singles.tile([P, D], bf)
        wpb_s = singles.tile([HD, D], bf)
        qt_s = singles.tile([P, 2, S], bf)
        kt_s = singles.tile([P, 2, S], bf)
        v_s = singles.tile([P, KT, HPC, HD + 1], bf)

        scr_s = singles.tile([P, QC], bf)

        xt_r = xT.rearrange("(t p) q -> p t q", p=P)
        wqk_r = w_qk.rearrange("(t p) s m -> p t s m", p=P)
        wv_r = w_v.rearrange("(t p) m -> p t m", p=P)
        # PE p-state warmup: the engine ramps to full clock only after ~3us
        # of busy time, so burn the initial DMA wait on dummy matmuls over a
        # memset scratch tile (no input dependencies)
        nc.vector.memset(scr_s[:, 0:P], 0.0)
        NWARM = int(os.environ.get("K_NWARM", "16"))
        for w in range(NWARM):
            ps_w = pmm.tile([P, QC], f32, tag="mm", name="ps_w")
            nc.tensor.matmul(ps_w[:, 0:P], lhsT=scr_s[:, 0:P],
                             rhs=scr_s[:, 0:P], start=True, stop=True)
        # critical-path loads first: chunk-0 QKV needs wqk + xT cols 0:512,
        # finely split so the first matmuls can start ASAP
        nc.sync.dma_start(out=wqk_s[:, 0:1], in_=wqk_r[:, 0:1])
        nc.scalar.dma_start(out=xT_s[:, 0:3, 0:HF], in_=xt_r[:, 0:3, 0:HF])
        nc.sync.dma_start(out=wqk_s[:, 1:3], in_=wqk_r[:, 1:3])
        nc.scalar.dma_start(out=xT_s[:, 3:CT, 0:HF], in_=xt_r[:, 3:CT, 0:HF])
        nc.sync.dma_start(out=wqk_s[:, 3:CT], in_=wqk_r[:, 3:CT])
        nc.scalar.dma_start(out=xT_s[:, 0:3, HF:QC], in_=xt_r[:, 0:3, HF:QC])
        nc.scalar.dma_start(out=xT_s[:, 3:CT, HF:QC], in_=xt_r[:, 3:CT, HF:QC])
        nc.gpsimd.dma_start(out=bq_s, in_=bq[:])
        nc.gpsimd.dma_start(out=mask_s, in_=mask[:])
        nc.gpsimd.dma_start(out=ident_s, in_=ident[:])
        nc.gpsimd.dma_start(out=wv_s[:, 0:3], in_=wv_r[:, 0:3])
        nc.gpsimd.dma_start(out=wv_s[:, 3:CT], in_=wv_r[:, 3:CT])
        for qc in range(1, NQC):
            nc.gpsimd.dma_start(out=xT_s[:, :, qc * QC:(qc + 1) * QC],
                                in_=xt_r[:, :, qc * QC:(qc + 1) * QC])
        nc.gpsimd.dma_start(out=wpa_s, in_=w_p[0:P, :])
        nc.gpsimd.dma_start(out=wpb_s, in_=w_p[P:P + HD, :])
        nc.gpsimd.memset(v_s[:, :, :, HD:HD + 1], 1.0)

        # head slices in the packed Q^T/K^T layout
        hsl = [slice(0, HD), slice(HD, P), slice(0, HD)]
        hslot = [0, 0, 1]
        # HW: matmuls writing one PSUM bank must share a partition base.
        # Bank 0 holds slots 0,1 (heads 0,2 - both base-0 operands); bank 1
        # holds slot 2 (head 1, whose operands sit at partition base 64).
        HORD = (0, 2, 1)  # head occupying each score-psum/pt slot
        SLOT = (0, 2, 1)  # slot holding each head (involution of HORD)

        # ---- QKV projection work items: (est_cols, fn) ----
        def g_kk(c, t0, t1):
            # K^T (heads 0/1) for k-tiles t0:t1
            lo, hi = t0 * P, t1 * P

            def f():
                ps_kk = pmm.tile([P, QC], f32, tag="mm", name="ps_kk")
                n = hi - lo
                for ct in range(CT):
                    nc.tensor.matmul(ps_kk[:, 0:n], lhsT=wqk_s[:, ct, 0, :],
                                     rhs=xT_s[:, ct, lo:hi],
                                     start=(ct == 0), stop=(ct == CT - 1))
                nc.vector.tensor_copy(out=kt_s[:, 0, lo:hi], in_=ps_kk[:, 0:n])
            return (6 * (hi - lo), f)

        def g_qq(c, half):
            qsl = slice(c * QC + half * HF, c * QC + (half + 1) * HF)

            def f():
                ps_qq = pmm.tile([P, QC], f32, tag="mm", name="ps_qq")
                for ct in range(CT):
                    nc.tensor.matmul(ps_qq[:, 0:HF], lhsT=wqk_s[:, ct, 1, :],
                                     rhs=xT_s[:, ct, qsl],
                                     start=(ct == 0), stop=(ct == CT - 1))
                nc.vector.tensor_scalar_add(out=qt_s[:, 0, qsl],
                                            in0=ps_qq[:, 0:HF],
                                            scalar1=bq_s[:, 0:1])
            return (6 * HF, f)

        def g_kq2(c, half):
            qsl = slice(c * QC + half * HF, c * QC + (half + 1) * HF)

            def f():
                ps_kq2 = pmm.tile([P, QC], f32, tag="mm", name="ps_kq2")
                for ct in range(CT):
                    nc.tensor.matmul(ps_kq2[:, 0:HF], lhsT=wqk_s[:, ct, 2, :],
                                     rhs=xT_s[:, ct, qsl],
                                     start=(ct == 0), stop=(ct == CT - 1))
                nc.vector.tensor_copy(out=kt_s[0:HD, 1, qsl],
                                      in_=ps_kq2[0:HD, 0:HF])
                # head2 Q lands in parts 64:128; bias-add, then repartition DMA
                q2st = np_pool.tile([P, QC], bf, tag="q2st", name="q2st")
                nc.vector.tensor_scalar_add(out=q2st[HD:P, 0:HF],
                                            in0=ps_kq2[HD:P, 0:HF],
                                            scalar1=bq_s[HD:P, 1:2])
                nc.sync.dma_start(out=qt_s[0:HD, 1, qsl], in_=q2st[HD:P, 0:HF])
            return (6 * HF, f)

        def g_v(kt):
            def f():
                ps_v = pmm.tile([P, QC], f32, tag="mm", name="ps_v")
                for ct in range(CT):
                    nc.tensor.matmul(ps_v[:, 0:HPC * HD],
                                     lhsT=xT_s[:, ct, kt * P:(kt + 1) * P],
                                     rhs=wv_s[:, ct, :],
                                     start=(ct == 0), stop=(ct == CT - 1))
                nc.vector.tensor_copy(
                    out=v_s[:, kt, :, 0:HD],
                    in_=ps_v[:, 0:HPC * HD].rearrange("p (h d) -> p h d", h=HPC))
            return (6 * HPC * HD, f)

        def qkv_items(c, split_late, coarse=False):
            # split_late: K-tiles/V per-ktile, paced into chunk c's own slots
            # (each is only needed at its kt slot). coarse: big groups that
            # pipeline with the staggered initial DMAs (chunk 0 only).
            if coarse:
                items = [g_kk(c, 4 * c, 4 * c + 4), g_qq(c, 0), g_qq(c, 1),
                         g_kq2(c, 0), g_kq2(c, 1)]
                items += [g_v(4 * c + i) for i in range(4)]
                return items, []
            items = [g_qq(c, 0), g_kq2(c, 0), g_qq(c, 1), g_kq2(c, 1)]
            late = []
            for i in range(4):
                if split_late:
                    late.append(g_kk(c, 4 * c + i, 4 * c + i + 1) + (i, True))
                    late.append(g_v(4 * c + i) + (i, False))
                else:
                    items.append(g_kk(c, 4 * c + i, 4 * c + i + 1))
                    items.append(g_v(4 * c + i))
            return items, late

        def emit_scores_exp(c, kt, pt_t):
            # scores (transposed S^T[k,q]) for all 3 heads in one psum tile,
            # exp'd in one ACT instruction per 256-col half into pt_t (SBUF).
            qs = c * QC
            scores_cols = 0
            exp_ns = 0.0
            for half in range(2):
                hbase = half * HF
                off = max(0, kt * P - qs - hbase)
                if off >= HF:
                    continue
                n = HF - off
                ps = ps3.tile([P, HPC, HF], f32, tag="ss", name="ps")
                if os.environ.get("K_NO_SCORES"):
                    continue
                for s, h in enumerate(HORD):
                    nc.tensor.matmul(
                        ps[:, s, 0:n],
                        lhsT=kt_s[hsl[h], hslot[h], kt * P:(kt + 1) * P],
                        rhs=qt_s[hsl[h], hslot[h],
                                 qs + hbase + off:qs + hbase + HF],
                        start=True, stop=True)
                if not os.environ.get("K_NO_EXP"):
                    nc.scalar.activation(out=pt_t[:, :, hbase + off:hbase + HF],
                                         in_=ps[:, :, 0:n], func=Exp,
                                         scale=0.125)
                if kt * P >= qs and not os.environ.get("K_NO_EXP"):
                    o = kt * P - qs - hbase  # diag block is inside one half
                    if 0 <= o < HF:
                        # zero the k>q triangle post-exp (Pool, SBUF only on
                        # HW); AV reads it 2 slots later, so no tail latency
                        for h in range(HPC):
                            nc.gpsimd.tensor_mul(
                                out=pt_t[:, h, hbase + o:hbase + o + P],
                                in0=pt_t[:, h, hbase + o:hbase + o + P],
                                in1=mask_s)
                scores_cols += 3 * n
                exp_ns += 3 * n * 0.833 + 370
            return scores_cols, exp_ns

        # AV accumulator: 12 chains (4 tiles x 3 heads) packed 7-per-bank
        # (73-f32 stride keeps each 65-f32 chain inside one 2KB bank) in TWO
        # independently rotating 1-bank tiles; the next chunk's memset then
        # only waits on the norms of the chains in its own bank
        def av_slot(pv, i, h):
            fl = 3 * i + h
            t, s = (0, fl) if fl < 7 else (1, fl - 7)
            o = s * (HD + 9)
            return pv[t][:, o:o + HD + 1]

        def emit_avs(c, kt, pv, pts, st_by_tile, use_act=False):
            # kt's AV contribution for every tile of the chunk with j >= kt;
            # chains that hit their diagonal finish (stop) and normalize.
            av_cols = 0
            import os as _os
            if _os.environ.get("K_NO_AV"):
                return 0
            for i in range(4):
                j = 4 * c + i
                if j < kt:
                    continue
                for h in range(HPC):
                    fl = 3 * i + h
                    nc.tensor.matmul(av_slot(pv, i, h),
                                     lhsT=pts[kt][:, SLOT[h], i * P:(i + 1) * P],
                                     rhs=v_s[:, kt, h, :],
                                     start=(kt == 0 and fl in (0, 7)),
                                     stop=(kt == j),
                                     skip_group_check=True)
                av_cols += HPC * (HD + 1)
                if kt == j:  # normalize tile j now (DVE; frees pav by chunk end)
                    st = st_by_tile[i] = {}
                    st['ao01'] = ao_pool.tile([P, 2, HD], bf, tag="ao01",
                                              name="ao01")
                    st['ao2'] = ao_pool.tile([P, P], bf, tag="ao2", name="ao2")
                    if c + 1 < NQC:  # padded: DMA-transpose reads all 128 cols
                        nc.gpsimd.memset(st['ao2'][:, HD:P], 0.0)
                    for h in range(HPC):
                        sl = av_slot(pv, i, h)
                        recip = np_pool.tile([P, 1], f32, tag="recip",
                                             name="recip")
                        nc.vector.reciprocal(out=recip, in_=sl[:, HD:HD + 1])
                        dst = (st['ao01'][:, h, :] if h < 2
                               else st['ao2'][:, 0:HD])
                        if use_act:  # ACT is idle after its last exp
                            nc.scalar.activation(
                                out=dst, in_=sl[:, 0:HD],
                                func=mybir.ActivationFunctionType.Copy,
                                scale=recip[:, 0:1])
                        else:
                            nc.vector.tensor_scalar_mul(
                                out=dst, in0=sl[:, 0:HD],
                                scalar1=recip[:, 0:1])
            return av_cols

        def tail_items(c, j, st, blockq, tick, tail=False, act_cp=False):
            # transpose (then self-enqueued projection) for q-tile j;
            # deferrable PE fill work. proj re-queues at the back so other
            # items run between transp's DVE copy and proj's read of it.
            def transp():
                aT01 = at_pool.tile([P, P], bf, tag="aT01", name="aT01")
                aT2 = at_pool.tile([P, P], bf, tag="aT2", name="aT2")
                if c + 1 < NQC and not os.environ.get("K_NO_DMAT"):
                    # deferred tiles: transpose on the DMA crossbar (no PE
                    # columns, no PSUM, no DVE copies); latency hidden by
                    # the +2-slot proj delay
                    nc.sync.dma_start_transpose(
                        out=aT01, in_=st['ao01'].rearrange("p h d -> p (h d)"))
                    nc.sync.dma_start_transpose(out=aT2, in_=st['ao2'])
                else:
                    cp = nc.scalar.copy if act_cp else nc.vector.tensor_copy
                    psT1 = pmm.tile([P, P], bf, tag="mm", name="psT1")
                    nc.tensor.transpose(
                        out=psT1, in_=st['ao01'].rearrange("p h d -> p (h d)"),
                        identity=ident_s)
                    cp(out=aT01, in_=psT1)
                    psT2 = pmm.tile([P, P], bf, tag="mm", name="psT2")
                    nc.tensor.transpose(out=psT2[0:HD, :],
                                        in_=st['ao2'][:, 0:HD],
                                        identity=ident_s)
                    cp(out=aT2[0:HD, :], in_=psT2[0:HD, :])
                st['aT01'], st['aT2'] = aT01, aT2[0:HD, :]
                lag = 2 if c + 1 < NQC else 1
                blockq.append((3 * 512, proj_half(0, 512), tick[0] + lag, BIGT))
                blockq.append((3 * 256, proj_half(512, 256), tick[0] + lag, BIGT))

            def proj_half(e0, en):
                def f():
                    pp = pmm.tile([P, QC], f32, tag="mm", name="pp")
                    nc.tensor.matmul(pp[:, 0:en], lhsT=st['aT01'],
                                     rhs=wpa_s[:, e0:e0 + en],
                                     start=True, stop=False)
                    nc.tensor.matmul(pp[:, 0:en], lhsT=st['aT2'][0:HD, :],
                                     rhs=wpb_s[:, e0:e0 + en],
                                     start=False, stop=True)
                    ob = outs_pool.tile([P, QC], bf, tag="ob", name="ob")
                    if tail:  # ACT copy + ACT-issued DMA: post-exp idle engine
                        nc.scalar.copy(out=ob[:, 0:en], in_=pp[:, 0:en])
                        nc.scalar.dma_start(
                            out=out_p[j * P:(j + 1) * P, e0:e0 + en],
                            in_=ob[:, 0:en])
                    else:
                        nc.vector.tensor_copy(out=ob[:, 0:en], in_=pp[:, 0:en])
                        nc.sync.dma_start(
                            out=out_p[j * P:(j + 1) * P, e0:e0 + en],
                            in_=ob[:, 0:en])
                return f

            tlag = int(os.environ.get("K_TLAG", "1")) if c == NQC - 1 else 1
            return [(2 * P, transp, tick[0] + tlag, BIGT)]

        # ---- main pipeline with deficit-paced fill work ----
        import os
        BIGT = 10 ** 9
        tick = [0]
        _cpn = os.environ.get("K_COLS_PER_NS", "1.8")
        if "," in _cpn:
            CPN_BY_CHUNK = [float(x) for x in _cpn.split(",")]
        else:
            CPN_BY_CHUNK = [float(_cpn)] * 4
        LATE_BONUS = float(os.environ.get("K_LATE_BONUS", "0.8"))
        AVLAG = int(os.environ.get("K_AVLAG", "2"))
        blockq = deque()

        # chunk 0: minimal head (k-tile 0 + q halves), rest paced in-loop
        for _, f in (g_kk(0, 0, 1), g_qq(0, 0), g_kq2(0, 0),
                     g_qq(0, 1), g_kq2(0, 1)):
            f()
        qkv_late = [g_kk(0, 1, 2) + (1, True), g_v(0) + (0, False),
                    g_kk(0, 2, 3) + (2, True), g_v(1) + (1, False),
                    g_kk(0, 3, 4) + (3, True), g_v(2) + (2, False),
                    g_v(3) + (3, False)]
        BIG = 10 ** 9
        NCH = int(os.environ.get("K_NCHUNKS", str(NQC)))
        for c in range(NCH):
            nkt = 4 * (c + 1)
            T0 = tick[0]
            for cols, f, rel, is_kk in reversed(qkv_late):
                # hard deadline: kk before its scores slot, v before its AVs
                due = T0 + rel + (1 if is_kk else AVLAG)
                blockq.appendleft((cols, f, 0, due))
            if c + 1 < NCH:
                qkv_due, qkv_late = qkv_items(c + 1, c + 1 == NQC - 1)
            else:
                qkv_due, qkv_late = [], []
            qkv_done = 0
            deficit = 0.0
            pts = []
            st_by_tile = {}
            # no memset: the first chain's start=True AV (fl 0 / fl 7,
            # emitted first per bank at kt=0) zeroes its whole 2KB bank via
            # the PSUM auto-zero region; every other chain accumulates onto
            # those zeros with start=False
            pvA = pav.tile([P, QC], f32, tag="av", name="pvA")
            pvB = pav.tile([P, QC], f32, tag="av", name="pvB")
            pv = (pvA, pvB)
            def fin_tile(kt2):
                # tile kt2's chains just finished (avs emitted): queue tail
                if os.environ.get("K_NO_TAILS"):
                    return
                if kt2 >= 4 * c:
                    i = kt2 - 4 * c
                    blockq.extend(tail_items(c, kt2, st_by_tile[i], blockq, tick,
                                             tail=(c == NQC - 1 and i == 3)))

            def pops(limit=None):
                nonlocal deficit, qkv_done
                want = (len(qkv_due) * (kt + 1)) // max(1, nkt - 1)
                while qkv_done < min(want, len(qkv_due)):
                    cols, f = qkv_due[qkv_done]
                    f()
                    qkv_done += 1
                    deficit -= cols
                n = 0
                held = []
                while blockq and deficit > 0 and (limit is None or n < limit):
                    item = blockq.popleft()
                    if item[2] > tick[0]:
                        held.append(item)
                        continue
                    item[1]()
                    deficit -= item[0]
                    n += 1
                for it in reversed(held):
                    blockq.appendleft(it)

            def force_due():
                # emit everything whose deadline has arrived, deficit or not
                nonlocal deficit
                due = [it for it in blockq if it[3] <= tick[0]]
                if due:
                    rest = [it for it in blockq if it[3] > tick[0]]
                    blockq.clear()
                    blockq.extend(rest)
                    for it in due:
                        it[1]()
                        deficit -= it[0]

            for kt in range(nkt):
                tick[0] += 1
                force_due()
                pt_t = ptp.tile([P, HPC, QC], bf, tag="pt", name="pt")
                pts.append(pt_t)
                if os.environ.get("K_AVFIRST") and kt >= AVLAG:
                    deficit -= emit_avs(c, kt - AVLAG, pv, pts, st_by_tile)
                sc, ens = emit_scores_exp(c, kt, pt_t)
                rate = CPN_BY_CHUNK[c]
                if c == NQC - 1 and kt >= nkt - int(os.environ.get("K_BWIN", "6")):
                    rate = rate + LATE_BONUS
                deficit += ens * rate - sc
                if kt >= AVLAG:  # AVs run AVLAG exps behind: no ACT waits
                    if not os.environ.get("K_AVFIRST"):
                        deficit -= emit_avs(c, kt - AVLAG, pv, pts, st_by_tile)
                    fin_tile(kt - AVLAG)
                pops(limit=int(os.environ.get("K_POPLIM", "99")))
            for kt2 in range(nkt - AVLAG, nkt):  # drain AV lag before next chunk
                tick[0] += 1
                force_due()
                emit_avs(c, kt2, pv, pts, st_by_tile,
                         use_act=(c == NQC - 1))
                fin_tile(kt2)
                pops(limit=int(os.environ.get("K_BPOP", "2")))
            while qkv_done < len(qkv_due):
                cols, f = qkv_due[qkv_done]
                f()
                qkv_done += 1

        while blockq:
            tick[0] += 1
            item = blockq.popleft()
            item[1]()

    nc.compile()
    return nc


def _prep_inputs(x, W_qkv, b_qkv, W_proj):
    """Build the 8 per-core input maps (all bf16 except biases)."""
    in_maps = []
    for cid in range(NCORES):
        b, g = divmod(cid, 4)
        hs = [g * HPC + i for i in range(HPC)]  # global head ids

        def wslice(kind, h):  # kind 0=q 1=k 2=v
            return W_qkv[:, kind * D + h * HD:(kind * D + (h + 1) * HD)]

        xT = np.ascontiguousarray(x[b].T).astype(_BF)

        w_qk = np.zeros((D, 3, P), dtype=np.float32)
        w_qk[:, 0, 0:HD] = wslice(1, hs[0])
        w_qk[:, 0, HD:P] = wslice(1, hs[1])
        w_qk[:, 1, 0:HD] = wslice(0, hs[0])
        w_qk[:, 1, HD:P] = wslice(0, hs[1])
        w_qk[:, 2, 0:HD] = wslice(1, hs[2])
        w_qk[:, 2, HD:P] = wslice(0, hs[2])

        w_v = np.concatenate([wslice(2, h) for h in hs], axis=1)

        bq = np.zeros((P, 2), dtype=np.float32)
        bq[0:HD, 0] = b_qkv[hs[0] * HD:(hs[0] + 1) * HD]
        bq[HD:P, 0] = b_qkv[hs[1] * HD:(hs[1] + 1) * HD]
        bq[HD:P, 1] = b_qkv[hs[2] * HD:(hs[2] + 1) * HD]

        w_p = np.concatenate([W_proj[h * HD:(h + 1) * HD, :] for h in hs], axis=0)

        mask = np.triu(np.ones((P, P), dtype=np.float32))
        ident = np.eye(P, dtype=np.float32)

        in_maps.append({
            "xT": xT,
            "w_qk": w_qk.astype(_BF),
            "w_v": w_v.astype(_BF),
            "bq": bq,
            "w_p": w_p.astype(_BF),
            "mask": mask.astype(_BF),
            "ident": ident.astype(_BF),
        })
    return in_maps


def _run(inputs, trace=False):
    from concourse.bass_utils import run_bass_kernel_spmd

    x = np.asarray(inputs["x"], dtype=np.float32)
    W_qkv = np.asarray(inputs["W_qkv"], dtype=np.float32)
    b_qkv = np.asarray(inputs["b_qkv"], dtype=np.float32)
    W_proj = np.asarray(inputs["W_proj"], dtype=np.float32)
    b_proj = np.asarray(inputs["b_proj"], dtype=np.float32)

    if "nc" not in _cache:
        _cache["nc"] = _build_nc()
    nc = _cache["nc"]

    in_maps = _prep_inputs(x, W_qkv, b_qkv, W_proj)
    res = run_bass_kernel_spmd(nc, in_maps, core_ids=list(range(NCORES)),
                               trace=trace)

    host_bias = b_proj + b_qkv[2 * D:3 * D] @ W_proj  # b_v folded through proj
    B = x.shape[0]
    out = np.zeros((B, S, D), dtype=np.float32)
    for cid in range(NCORES):
        b = cid // 4
        out[b] += res.results[cid]["out_p"].astype(np.float32)
    out += host_bias
    return out, res


def kernel(x, W_qkv, b_qkv, W_proj, b_proj):
    out, _ = _run({"x": x, "W_qkv": W_qkv, "b_qkv": b_qkv,
                   "W_proj": W_proj, "b_proj": b_proj})
    return out
